# revision 20
# baseline (speedup 1.0000x reference)
"""Distributed attention kernel for 8 TRN2 NeuronCores.

Problem: x[2,2048,1024] -> qkv proj -> 16-head attention (softmax then /scale
quirk) -> out proj + bias.

Sharding: core c handles heads {2c, 2c+1} for BOTH batches (head-parallel).
Attention outputs are redistributed with four 8-core AllToAlls (one per
(batch, head)) in bf16, then each core projects one (batch, s/4) output slice
against the full (host-row-permuted) w_out; the mirror-half projection is
computed on stale slots and discarded by the host (SPMD artifact).

Schedule (single fused pipeline, no blocking drains):
  - Weights/x DMA descriptors issue first thing, split across the Sync and
    Vector queues (descriptor issue is ~620ns each and was the old start
    bottleneck); w_qkv and w_out are host-packed into single [128, K*cols]
    tiles so each is one descriptor.
  - Batch-0 QKV runs as fine-grained filler inside batch-0's attention units
    after a minimal 3-chain pre-drain; batch-1 QKV weaves into unit (0,1);
    projection A (phase-split by a2a arrival: K-chunks 0-3 / 4-7) weaves
    into units (1,0)/(1,1); projection B phase 1 weaves into unit (1,1); only
    projection B phase 2 (~4us) plus one 512KB-sent bf16 a2a remains in the
    tail.  Need-gated pulls keep the in-order PE queue from ever stalling on
    an unready operand.
  - Normalization: PSUM evacuation on DVE, reciprocal_approx_fast (the exact
    DVE reciprocal was 6.5us per [1,1024] on one partition), gpsimd
    partition-broadcast, then one scalar_tensor_tensor that scales by
    sqrt(dim_head) and writes the bf16 stage tile directly.

All matmuls run as float32r (fp32 storage, reduced-precision PE mode, 1
cycle/row at N=512).  Every matmul is padded to the full 128x128 array
(zero-padded per-head q tiles, zero-padded v columns): partial-array matmuls
don't register as busy for the HAM clock gate, which otherwise holds the PE
at 1.2 GHz instead of 2.4 GHz.  Do NOT mix bf16 and f32r matmuls in this
kernel - that combination produced nondeterministic weight corruption on
hardware (bf16 is used only for DMA'd a2a payloads, converted back to f32r
on DVE before any matmul touches them).
"""

import numpy as np

S = 2048          # sequence length
D = 1024          # model dim
NH = 16           # total heads
DH = 64           # head dim
HPC = 2           # heads per core
NCORES = 8
KC = 8            # k-chunks of D (128 each)
QH = 2            # q halves (1024 each) per attention unit
SCALE_INV = 8.0   # 1 / (DH ** -0.5)

_CACHE = {}


def _ensure_paths():
    import sys
    for p in ("/opt/trn_rl_repo", "/root/.axon_site"):
        if p not in sys.path:
            sys.path.insert(0, p)


class Weave:
    """Ordered queue of step generators with optional emission gates."""

    def __init__(self):
        self.q = []

    def add(self, gen, gate=None):
        self.q.append((gen, gate))

    def pull(self, n=1):
        done = 0
        while done < n and self.q:
            gen, gate = self.q[0]
            if gate is not None and not gate():
                return done
            try:
                next(gen)
                done += 1
            except StopIteration:
                self.q.pop(0)
        return done

    def pull_until(self, cond):
        while not cond():
            if self.pull(1) == 0:
                raise RuntimeError("weave starved while a need is unmet")


def _build_nc(debug_taps=False):
    _ensure_paths()
    from contextlib import ExitStack
    import concourse.bass as bass
    import concourse.mybir as mybir
    import concourse.tile as tile
    from concourse import bacc
    from concourse.masks import make_identity

    f32 = mybir.dt.float32
    f32r = mybir.dt.float32r
    bf16 = mybir.dt.bfloat16
    EXP = mybir.ActivationFunctionType.Exp

    nc = bacc.Bacc(None)
    xT_ext = nc.declare_dram_parameter("xT", [2, KC, 128, S], f32r, isOutput=False)
    wq_ext = nc.declare_dram_parameter("w_qkv", [128, KC * 3 * HPC * DH], f32r, isOutput=False)
    wo_ext = nc.declare_dram_parameter("w_out", [128, KC * D], f32r, isOutput=False)
    bout_ext = nc.declare_dram_parameter("b_out", [D], f32, isOutput=False)
    outA_ext = nc.declare_dram_parameter("outA", [512, D], f32, isOutput=True)
    outB_ext = nc.declare_dram_parameter("outB", [512, D], f32, isOutput=True)
    dbg = {}
    if debug_taps:
        dbg["qT"] = nc.declare_dram_parameter("dbg_qT", [128, S], f32, isOutput=True)
        dbg["kT"] = nc.declare_dram_parameter("dbg_kT", [128, S], f32, isOutput=True)
        dbg["ot"] = nc.declare_dram_parameter("dbg_ot", [DH + 1, 1024], f32, isOutput=True)
        dbg["stage"] = nc.declare_dram_parameter("dbg_stage", [128, S], f32, isOutput=True)
        dbg["g"] = nc.declare_dram_parameter("dbg_g", [KC, 128, 512], f32, isOutput=True)
        dbg["rc"] = nc.declare_dram_parameter("dbg_rc", [1, 1024], f32, isOutput=True)
        dbg["rcA"] = nc.declare_dram_parameter("dbg_rcA", [1, 1024], f32, isOutput=True)
        dbg["stB"] = nc.declare_dram_parameter("dbg_stB", [DH, 1024], f32, isOutput=True)
        dbg["bc"] = nc.declare_dram_parameter("dbg_bc", [DH, 1024], f32, isOutput=True)

    with tile.TileContext(nc) as tc, ExitStack() as ctx:
        ctx.enter_context(
            nc.allow_low_precision(reason="float32r is fp32-width storage; a2a in bf16")
        )
        const = ctx.enter_context(tc.tile_pool(name="const", bufs=1))
        qk_pool = ctx.enter_context(tc.tile_pool(name="qk", bufs=6))
        vt_pool = ctx.enter_context(tc.tile_pool(name="vt", bufs=1))
        vo_pool = ctx.enter_context(tc.tile_pool(name="vo", bufs=25))
        st_pool = ctx.enter_context(tc.tile_pool(name="st", bufs=2))
        ot_pool = ctx.enter_context(tc.tile_pool(name="ot", bufs=2))
        rc_pool = ctx.enter_context(tc.tile_pool(name="rcp", bufs=2))
        bc_pool = ctx.enter_context(tc.tile_pool(name="bc", bufs=2))
        stage_pool = ctx.enter_context(tc.tile_pool(name="stg", bufs=1))

        ps_a = ctx.enter_context(tc.tile_pool(name="psA", bufs=2, space="PSUM"))
        ps_lt = ctx.enter_context(tc.tile_pool(name="psLT", bufs=2, space="PSUM"))
        ps_ot = ctx.enter_context(tc.tile_pool(name="psOT", bufs=1, space="PSUM"))
        dram = ctx.enter_context(tc.tile_pool(name="dram", bufs=1, space="DRAM"))

        a2a_in = dram.tile([HPC, NCORES, DH, 512], f32r, tag="a2a_in", name="a2a_in")
        a2a_out = dram.tile([2, HPC, NCORES, DH, 512], f32r, tag="a2a_out", name="a2a_out")

        weave = Weave()
        prog = {}
        for b in range(2):
            prog[("k", b)] = 0     # kT chains done (of 4 nkk)
            prog[("q", b)] = 0     # qT chains done (of 4 nkk)
            prog[("vtr", b)] = 0   # vo transposes done (of 16)

        qT = {}
        kT = {}
        vo = {}

        # ---- phase-A scope: x tiles + w_qkv ----
        pa = ExitStack()
        xt_pool = pa.enter_context(tc.tile_pool(name="xt", bufs=33))
        wq_pool = pa.enter_context(tc.tile_pool(name="wq", bufs=1))

        # DMA descriptor issue, split across engines.  sync: w_qkv + b0
        # nkk0/nkk1; vector: b0 nkk2/nkk3 (before any vector compute).
        wq_sb = wq_pool.tile([128, KC * 3 * HPC * DH], f32r, tag="wq", name="wq")
        nc.sync.dma_start(out=wq_sb, in_=wq_ext.ap())

        xts = {}  # (b, k, nkk) -> [128, 512] tile

        def issue_xt(eng, b, k, nkk):
            t = xt_pool.tile([128, 512], f32r, tag="xt", name=f"xt{b}_{k}_{nkk}")
            eng.dma_start(out=t, in_=xT_ext[b, k][:, nkk * 512:(nkk + 1) * 512])
            xts[(b, k, nkk)] = t

        for nkk in (0, 1):
            for k in range(KC):
                issue_xt(nc.sync, 0, k, nkk)
        for nkk in (2, 3):
            for k in range(KC):
                issue_xt(nc.scalar, 0, k, nkk)

        # ---- constants (after the vector-queue descriptors) ----
        ident_scr = const.tile([128, 128], f32, tag="identS", name="ident_scr")
        make_identity(nc, ident_scr)
        ident = const.tile([128, 128], f32r, tag="ident", name="ident")
        nc.vector.tensor_copy(ident, ident_scr)
        ones2 = const.tile([128, HPC, 1], f32, tag="ones2", name="ones2")
        nc.vector.memset(ones2, 1.0)
        zeros2 = const.tile([128, HPC, 128 - DH - 1], f32, tag="zeros2", name="zeros2")
        nc.vector.memset(zeros2, 0.0)
        # memset on f32r tiles is invalid ISA; zero qT halves via an f32 scratch
        zpad = const.tile([DH, 512], f32, tag="zpad", name="zpad")
        nc.vector.memset(zpad, 0.0)

        def qkv_gen(b):
            """Chain order: k:0 q:0 q:1 v:0+tr k:1 v:1+tr k:2 k:3 q:2 q:3
            v:2+tr v:3+tr.  One yield per matmul / copy group."""
            qT[b] = [
                qk_pool.tile([128, S], f32r, tag="qk", name=f"qT{b}_{h}")
                for h in range(HPC)
            ]
            kT[b] = qk_pool.tile([128, S], f32r, tag="qk", name=f"kT{b}")
            vo[b] = [None] * (S // 128)
            for h in range(HPC):
                r0 = 64 * (1 - h)
                for cq in range(4):
                    nc.vector.tensor_copy(
                        qT[b][h][r0:r0 + 64, cq * 512:(cq + 1) * 512], zpad
                    )
            vT = [None]

            def chain(kind, nkk):
                # kind: 'q' (c0=0), 'k' (c0=1), 'v' (c0=2) column block
                c0 = {"q": 0, "k": 1, "v": 2}[kind] * 128
                ps = ps_a.tile([128, 512], f32, tag="psA", name=f"qkv{b}_{kind}{nkk}")
                for k in range(KC):
                    nc.tensor.matmul(
                        ps,
                        lhsT=wq_sb[:, k * 384 + c0:k * 384 + c0 + 128],
                        rhs=xts[(b, k, nkk)],
                        start=(k == 0),
                        stop=(k == KC - 1),
                    )
                    yield
                cols = slice(nkk * 512, (nkk + 1) * 512)
                if kind == "q":
                    for h in range(HPC):
                        nc.vector.tensor_copy(
                            qT[b][h][64 * h:64 * h + 64, cols], ps[64 * h:64 * h + 64, :]
                        )
                    prog[("q", b)] += 1
                elif kind == "k":
                    nc.vector.tensor_copy(kT[b][:, cols], ps)
                    prog[("k", b)] += 1
                else:
                    if vT[0] is None:
                        vT[0] = vt_pool.tile([128, S], f32r, tag="vt", name=f"vT{b}")
                    nc.vector.tensor_copy(vT[0][:, cols], ps)
                yield

            def vtrans(nkk):
                for sc in range(nkk * 4, nkk * 4 + 4):
                    vps = ps_a.tile([128, 128], f32r, tag="psA", name=f"vps{b}_{sc}")
                    nc.tensor.transpose(vps, vT[0][:, sc * 128:(sc + 1) * 128], ident)
                    vt = vo_pool.tile([128, HPC, 128], f32r, tag="vo", name=f"vo{b}_{sc}")
                    nc.vector.tensor_copy(
                        vt[:, :, 0:DH], vps.rearrange("p (h d) -> p h d", h=HPC)
                    )
                    nc.vector.tensor_copy(vt[:, :, DH:DH + 1], ones2)
                    nc.vector.tensor_copy(vt[:, :, DH + 1:], zeros2)
                    vo[b][sc] = vt
                    prog[("vtr", b)] += 1
                    yield

            yield from chain("k", 0)
            yield from chain("q", 0)
            yield from chain("q", 1)
            yield from chain("v", 0)
            yield from vtrans(0)
            yield from chain("k", 1)
            yield from chain("v", 1)
            yield from vtrans(1)
            yield from chain("k", 2)
            yield from chain("k", 3)
            yield from chain("q", 2)
            yield from chain("q", 3)
            yield from chain("v", 2)
            yield from vtrans(2)
            yield from chain("v", 3)
            yield from vtrans(3)

        state = {"pos": (-1, -1)}  # (global unit index 0..3, qh)

        def pos_gate(u, qh):
            return lambda: state["pos"] >= (u, qh)

        def attention_unit(uidx, b, hh, stage, hooks=None, rate=3):
            h0 = hh * DH
            NK = S // 128
            for qh in range(QH):
                state["pos"] = (uidx, qh)
                q0 = qh * 1024
                weave.pull_until(lambda: prog[("q", b)] >= 2 * (qh + 1))

                def sv(k, st):
                    for half in range(2):
                        nc.tensor.matmul(
                            outT[:, half * 512:(half + 1) * 512],
                            lhsT=vo[b][k][:, hh, :],
                            rhs=st[:, half * 512:(half + 1) * 512],
                            start=(k == 0),
                            stop=(k == NK - 1),
                        )

                outT = ps_ot.tile([128, 1024], f32, tag="psOT", name=f"outT{b}_{hh}_{qh}")
                pending = None
                for k in range(NK):
                    weave.pull_until(lambda: prog[("k", b)] > k // 4)
                    lt = ps_lt.tile([128, 1024], f32, tag="psLT", name=f"lt{b}_{hh}_{qh}_{k}")
                    for half in range(2):
                        nc.tensor.matmul(
                            lt[:, half * 512:(half + 1) * 512],
                            lhsT=kT[b][:, k * 128:(k + 1) * 128],
                            rhs=qT[b][hh][:, q0 + half * 512:q0 + (half + 1) * 512],
                            start=True,
                            stop=True,
                        )
                    st = st_pool.tile([128, 1024], f32r, tag="st", name=f"st{b}_{hh}_{qh}_{k}")
                    nc.scalar.activation(st, lt, EXP)
                    if pending is not None:
                        weave.pull_until(lambda: prog[("vtr", b)] > pending[0])
                        sv(*pending)
                    pending = (k, st)
                    weave.pull(rate)
                weave.pull_until(lambda: prog[("vtr", b)] > pending[0])
                sv(*pending)
                # normalize this q-half straight into the stage tile
                ot_sb = ot_pool.tile([DH, 1024], f32, tag="ot", name=f"ot{b}_{hh}_{qh}")
                nc.vector.tensor_copy(ot_sb, outT[0:DH, :])
                sums = rc_pool.tile([1, 1024], f32, tag="rcp", name=f"sums{b}_{hh}_{qh}")
                nc.vector.tensor_copy(sums, outT[DH:DH + 1, :])
                recip = rc_pool.tile([1, 1024], f32, tag="rcp", name=f"rcp{b}_{hh}_{qh}")
                # approx_fast needs a partition-0-aligned input (an offset-64
                # input returned garbage); ~5x faster than exact reciprocal
                nc.vector.reciprocal_approx_fast(out=recip, in_=sums)
                if dbg and b == 0 and hh == 0 and qh == 0:
                    nc.sync.dma_start(out=dbg["ot"][0:DH], in_=ot_sb)
                    nc.sync.dma_start(out=dbg["ot"][DH:DH + 1], in_=sums)
                bc_sb = bc_pool.tile([DH, 1024], f32, tag="bc", name=f"bc{b}_{hh}_{qh}")
                nc.gpsimd.partition_broadcast(bc_sb, recip)
                if dbg and b == 0 and hh == 0 and qh == 0:
                    nc.sync.dma_start(out=dbg["rc"].ap(), in_=recip)
                    nc.sync.dma_start(out=dbg["bc"].ap(), in_=bc_sb)
                nc.vector.scalar_tensor_tensor(
                    out=stage[h0:h0 + DH, q0:q0 + 1024],
                    in0=ot_sb,
                    scalar=SCALE_INV,
                    in1=bc_sb,
                    op0=mybir.AluOpType.mult,
                    op1=mybir.AluOpType.mult,
                )
                if hooks and qh in hooks:
                    for fn in hooks[qh]:
                        fn()

        def a2a_fire(b, hh, stage):
            h0 = hh * DH
            for qq in range(4):
                nc.gpsimd.dma_start(
                    out=a2a_in[hh, 4 * b + qq],
                    in_=stage[h0:h0 + DH, qq * 512:(qq + 1) * 512],
                )
            nc.gpsimd.collective_compute(
                "AllToAll",
                mybir.AluOpType.bypass,
                replica_groups=[list(range(NCORES))],
                ins=[a2a_in[hh].opt()],
                outs=[a2a_out[b, hh].opt()],
            )

        # ---------- batch 0 ----------
        weave.add(qkv_gen(0))
        weave.add(qkv_gen(1), gate=pos_gate(0, 0))
        # dense pre-drain of ALL of batch-0 QKV: an ~85-95%-busy PE burst here
        # is what convinces the HAM clock gate to lift the PE to 2.4 GHz by
        # ~t50; a sparser woven start left the whole kernel at 1.2 GHz
        weave.pull_until(lambda: prog[("vtr", 0)] >= 16)

        stage0 = stage_pool.tile([HPC * DH, S], f32r, tag="stg", name="stg0")

        # batch-1 x descriptors on the idle Scalar queue (first exp ~t50)
        for nkk in range(4):
            for k in range(KC):
                issue_xt(nc.scalar, 1, k, nkk)

        attention_unit(0, 0, 0, stage0)
        a2a_fire(0, 0, stage0)
        attention_unit(1, 0, 1, stage0)
        a2a_fire(0, 1, stage0)
        if dbg:
            nc.sync.dma_start(out=dbg["qT"].ap(), in_=qT[0][0][:].bitcast(f32))
            nc.sync.dma_start(out=dbg["kT"].ap(), in_=kT[0][:].bitcast(f32))
            nc.sync.dma_start(out=dbg["stage"].ap(), in_=stage0[:].bitcast(f32))

        # ---------- close phase-A scope, open projection pools ----------
        weave.pull_until(lambda: prog[("vtr", 1)] >= 16)  # finish b1 qkv emission
        pa.close()
        wo_pool = ctx.enter_context(tc.tile_pool(name="wo", bufs=1))
        g_pool = ctx.enter_context(tc.tile_pool(name="g", bufs=10))
        y_pool = ctx.enter_context(tc.tile_pool(name="y", bufs=9))
        bias_pool = ctx.enter_context(tc.tile_pool(name="bias", bufs=1))

        wo_sb = wo_pool.tile([128, KC * D], f32r, tag="wo", name="wo")
        nc.sync.dma_start(out=wo_sb, in_=wo_ext.ap())
        bias_sb = bias_pool.tile([128, D], f32, tag="bias", name="bias_sb")
        bias_ap = bout_ext.ap()
        bias_bcast = bass.AP(
            tensor=bias_ap.tensor,
            offset=bias_ap.offset,
            ap=[[0, 128]] + [list(p) for p in bias_ap.ap],
        )
        nc.sync.dma_start(out=bias_sb, in_=bias_bcast)

        gf_tiles = {}  # (half, k) -> f32r tile

        def g_dma(half, ks):
            for k in ks:
                hh, cc = (0, k) if k < 4 else (1, k - 4)
                t = g_pool.tile([128, 512], f32r, tag="g", name=f"g{half}_{k}")
                nc.gpsimd.dma_start(
                    out=t,
                    in_=a2a_out[half, hh, 2 * cc:2 * cc + 2].rearrange(
                        "s d c -> (s d) c"
                    ),
                )
                gf_tiles[(half, k)] = t

        g_dma(0, range(KC))  # k0-3 dep a2a(0,0) done; k4-7 dep a2a(0,1)

        y_sb = {}  # (half, sc, nk) -> [128, 512] tile

        def proj_phase(half, phase, oext):
            for sc in range(4):
                for nk in range(2):
                    yps = ps_a.tile([128, 512], f32, tag="psA", name=f"yps{half}_{phase}_{sc}_{nk}")
                    for kk in range(4):
                        k = phase * 4 + kk
                        nc.tensor.matmul(
                            yps,
                            lhsT=gf_tiles[(half, k)][:, sc * 128:(sc + 1) * 128],
                            rhs=wo_sb[:, k * D + nk * 512:k * D + (nk + 1) * 512],
                            start=(kk == 0),
                            stop=(kk == 3),
                        )
                        yield
                    if phase == 0:
                        y = y_pool.tile([128, 512], f32, tag="y", name=f"y{half}_{sc}_{nk}")
                        nc.vector.tensor_add(y, yps, bias_sb[:, nk * 512:(nk + 1) * 512])
                        y_sb[(half, sc, nk)] = y
                    else:
                        y = y_sb[(half, sc, nk)]
                        nc.vector.tensor_add(y, y, yps)
                        nc.sync.dma_start(
                            out=oext[sc * 128:(sc + 1) * 128, nk * 512:(nk + 1) * 512],
                            in_=y,
                        )
                    yield

        weave.add(proj_phase(0, 0, outA_ext), gate=pos_gate(2, 1))
        weave.add(proj_phase(0, 1, outA_ext), gate=pos_gate(3, 0))
        weave.add(proj_phase(1, 0, outB_ext), gate=pos_gate(3, 1))

        # ---------- batch 1 ----------
        stage1 = stage_pool.tile([HPC * DH, S], f32r, tag="stg", name="stg1")
        attention_unit(2, 1, 0, stage1)
        if dbg:
            for k in range(KC):
                nc.sync.dma_start(out=dbg["g"][k], in_=gf_tiles[(0, k)][:].bitcast(f32))
        a2a_fire(1, 0, stage1)
        g_dma(1, range(4))
        attention_unit(3, 1, 1, stage1)
        a2a_fire(1, 1, stage1)
        g_dma(1, range(4, 8))

        # ---------- tail: projection B ----------
        state["pos"] = (4, 0)
        while weave.pull(64):
            pass
        # dummy full-array matmuls (no consumer) keep the PE busy through the
        # last AllToAll so the HAM clock stays at 2.4 GHz for phase 2
        for w in range(48):
            wps = ps_lt.tile([128, 512], f32, tag="psLT", name=f"warm{w}")
            nc.tensor.matmul(
                wps,
                lhsT=kT[1][:, 0:128],
                rhs=qT[1][0][:, 0:512],
                start=True,
                stop=True,
            )
        for _ in proj_phase(1, 1, outB_ext):
            pass

    nc.finalize()
    return nc


def _prep_in_maps(x, w_qkv, w_out, b_out):
    x = np.ascontiguousarray(x, dtype=np.float32)
    w_qkv = np.ascontiguousarray(w_qkv, dtype=np.float32)
    w_out = np.ascontiguousarray(w_out, dtype=np.float32)
    b_out = np.ascontiguousarray(b_out, dtype=np.float32)

    xT = np.ascontiguousarray(
        np.stack([x[0].T, x[1].T]).reshape(2, KC, 128, S)
    )
    # arrival order after the per-(batch, head) AllToAll: call h delivers
    # head (2c+h) for c=0..7; stacked [call0 (512 rows), call1 (512 rows)].
    perm = []
    for h in range(HPC):
        for c in range(NCORES):
            base = 128 * c + DH * h
            perm.extend(range(base, base + DH))
    # packed [128, KC, D] so the full w_out is one DMA descriptor
    wo = np.ascontiguousarray(
        w_out[np.array(perm)].reshape(KC, 128, D).transpose(1, 0, 2).reshape(128, KC * D)
    )
    in_maps = []
    for c in range(NCORES):
        c0 = c * HPC * DH
        shard = np.concatenate(
            [
                w_qkv[:, c0:c0 + 128],
                w_qkv[:, D + c0:D + c0 + 128],
                w_qkv[:, 2 * D + c0:2 * D + c0 + 128],
            ],
            axis=1,
        )
        # packed [128, KC, 384] so the per-core w_qkv slice is one descriptor
        wq = np.ascontiguousarray(
            shard.reshape(KC, 128, 384).transpose(1, 0, 2).reshape(128, KC * 384)
        )
        in_maps.append(
            {
                "xT": xT,
                "w_qkv": wq,
                "w_out": wo,
                "b_out": b_out,
            }
        )
    return in_maps


def _run(x, w_qkv, w_out, b_out, trace=False, debug_taps=False):
    _ensure_paths()
    from concourse.bass_utils import run_bass_kernel_spmd

    key = "nc_dbg" if debug_taps else "nc"
    if key not in _CACHE:
        _CACHE[key] = _build_nc(debug_taps=debug_taps)
    nc = _CACHE[key]
    in_maps = _prep_in_maps(x, w_qkv, w_out, b_out)
    res = run_bass_kernel_spmd(nc, in_maps, list(range(NCORES)), trace=trace)
    out = np.empty((2, S, D), dtype=np.float32)
    for c in range(NCORES):
        b, q = c // 4, c % 4
        key = "outA" if b == 0 else "outB"
        out[b, 512 * q:512 * (q + 1), :] = res.results[c][key]
    return out, res


def kernel(x, w_qkv, w_out, b_out):
    out, _ = _run(x, w_qkv, w_out, b_out, trace=False)
    return out


# revision 21
# speedup vs baseline: 1.0968x; 1.0968x over previous
"""Distributed attention kernel for 8 TRN2 NeuronCores.

Problem: x[2,2048,1024] -> qkv proj -> 16-head attention (softmax then /scale
quirk) -> out proj + bias.

Sharding: core c handles heads {2c, 2c+1} for BOTH batches (head-parallel).
Attention outputs are redistributed with four 8-core AllToAlls (one per
(batch, head)) in bf16, then each core projects one (batch, s/4) output slice
against the full (host-row-permuted) w_out; the mirror-half projection is
computed on stale slots and discarded by the host (SPMD artifact).

Schedule (single fused pipeline, no blocking drains):
  - Weights/x DMA descriptors issue first thing, split across the Sync and
    Vector queues (descriptor issue is ~620ns each and was the old start
    bottleneck); w_qkv and w_out are host-packed into single [128, K*cols]
    tiles so each is one descriptor.
  - Batch-0 QKV runs as fine-grained filler inside batch-0's attention units
    after a minimal 3-chain pre-drain; batch-1 QKV weaves into unit (0,1);
    projection A (phase-split by a2a arrival: K-chunks 0-3 / 4-7) weaves
    into units (1,0)/(1,1); projection B phase 1 weaves into unit (1,1); only
    projection B phase 2 (~4us) plus one 512KB-sent bf16 a2a remains in the
    tail.  Need-gated pulls keep the in-order PE queue from ever stalling on
    an unready operand.
  - Normalization: PSUM evacuation on DVE, reciprocal_approx_fast (the exact
    DVE reciprocal was 6.5us per [1,1024] on one partition), gpsimd
    partition-broadcast, then one scalar_tensor_tensor that scales by
    sqrt(dim_head) and writes the bf16 stage tile directly.

All matmuls run as float32r (fp32 storage, reduced-precision PE mode, 1
cycle/row at N=512).  Every matmul is padded to the full 128x128 array
(zero-padded per-head q tiles, zero-padded v columns): partial-array matmuls
don't register as busy for the HAM clock gate, which otherwise holds the PE
at 1.2 GHz instead of 2.4 GHz.  Do NOT mix bf16 and f32r matmuls in this
kernel - that combination produced nondeterministic weight corruption on
hardware (bf16 is used only for DMA'd a2a payloads, converted back to f32r
on DVE before any matmul touches them).
"""

import numpy as np

S = 2048          # sequence length
D = 1024          # model dim
NH = 16           # total heads
DH = 64           # head dim
HPC = 2           # heads per core
NCORES = 8
KC = 8            # k-chunks of D (128 each)
QH = 2            # q halves (1024 each) per attention unit
SCALE_INV = 8.0   # 1 / (DH ** -0.5)

_CACHE = {}


def _ensure_paths():
    import sys
    for p in ("/opt/trn_rl_repo", "/root/.axon_site"):
        if p not in sys.path:
            sys.path.insert(0, p)


class Weave:
    """Ordered queue of step generators with optional emission gates."""

    def __init__(self):
        self.q = []

    def add(self, gen, gate=None):
        self.q.append((gen, gate))

    def pull(self, n=1):
        done = 0
        while done < n and self.q:
            gen, gate = self.q[0]
            if gate is not None and not gate():
                return done
            try:
                next(gen)
                done += 1
            except StopIteration:
                self.q.pop(0)
        return done

    def pull_until(self, cond):
        while not cond():
            if self.pull(1) == 0:
                raise RuntimeError("weave starved while a need is unmet")


def _build_nc(debug_taps=False):
    _ensure_paths()
    from contextlib import ExitStack
    import concourse.bass as bass
    import concourse.mybir as mybir
    import concourse.tile as tile
    from concourse import bacc
    from concourse.masks import make_identity

    f32 = mybir.dt.float32
    f32r = mybir.dt.float32r
    bf16 = mybir.dt.bfloat16
    EXP = mybir.ActivationFunctionType.Exp

    nc = bacc.Bacc(None)
    xT_ext = nc.declare_dram_parameter("xT", [2, KC, 128, S], f32r, isOutput=False)
    wq_ext = nc.declare_dram_parameter("w_qkv", [128, KC * 3 * HPC * DH], f32r, isOutput=False)
    wo_ext = nc.declare_dram_parameter("w_out", [128, KC * D], f32r, isOutput=False)
    bout_ext = nc.declare_dram_parameter("b_out", [D], f32, isOutput=False)
    outA_ext = nc.declare_dram_parameter("outA", [512, D], f32, isOutput=True)
    outB_ext = nc.declare_dram_parameter("outB", [512, D], f32, isOutput=True)
    dbg = {}
    if debug_taps:
        dbg["qT"] = nc.declare_dram_parameter("dbg_qT", [128, S], f32, isOutput=True)
        dbg["kT"] = nc.declare_dram_parameter("dbg_kT", [128, S], f32, isOutput=True)
        dbg["ot"] = nc.declare_dram_parameter("dbg_ot", [DH + 1, 1024], f32, isOutput=True)
        dbg["stage"] = nc.declare_dram_parameter("dbg_stage", [128, S], f32, isOutput=True)
        dbg["g"] = nc.declare_dram_parameter("dbg_g", [KC, 128, 512], f32, isOutput=True)
        dbg["rc"] = nc.declare_dram_parameter("dbg_rc", [1, 1024], f32, isOutput=True)
        dbg["rcA"] = nc.declare_dram_parameter("dbg_rcA", [1, 1024], f32, isOutput=True)
        dbg["stB"] = nc.declare_dram_parameter("dbg_stB", [DH, 1024], f32, isOutput=True)
        dbg["bc"] = nc.declare_dram_parameter("dbg_bc", [DH, 1024], f32, isOutput=True)

    with tile.TileContext(nc) as tc, ExitStack() as ctx:
        ctx.enter_context(
            nc.allow_low_precision(reason="float32r is fp32-width storage; a2a in bf16")
        )
        const = ctx.enter_context(tc.tile_pool(name="const", bufs=1))
        qk_pool = ctx.enter_context(tc.tile_pool(name="qk", bufs=6))
        vt_pool = ctx.enter_context(tc.tile_pool(name="vt", bufs=1))
        vo_pool = ctx.enter_context(tc.tile_pool(name="vo", bufs=25))
        st_pool = ctx.enter_context(tc.tile_pool(name="st", bufs=2))
        ot_pool = ctx.enter_context(tc.tile_pool(name="ot", bufs=2))
        rc_pool = ctx.enter_context(tc.tile_pool(name="rcp", bufs=2))
        bc_pool = ctx.enter_context(tc.tile_pool(name="bc", bufs=2))
        stage_pool = ctx.enter_context(tc.tile_pool(name="stg", bufs=1))

        ps_a = ctx.enter_context(tc.tile_pool(name="psA", bufs=2, space="PSUM"))
        ps_lt = ctx.enter_context(tc.tile_pool(name="psLT", bufs=2, space="PSUM"))
        ps_ot = ctx.enter_context(tc.tile_pool(name="psOT", bufs=1, space="PSUM"))
        dram = ctx.enter_context(tc.tile_pool(name="dram", bufs=1, space="DRAM"))

        a2a_in = dram.tile([HPC, NCORES, DH, 512], f32r, tag="a2a_in", name="a2a_in")
        a2a_out = dram.tile([2, HPC, NCORES, DH, 512], f32r, tag="a2a_out", name="a2a_out")

        weave = Weave()
        prog = {}
        for b in range(2):
            prog[("k", b)] = 0     # kT chains done (of 4 nkk)
            prog[("q", b)] = 0     # qT chains done (of 4 nkk)
            prog[("vtr", b)] = 0   # vo transposes done (of 16)

        qT = {}
        kT = {}
        vo = {}

        # ---- phase-A scope: x tiles + w_qkv ----
        pa = ExitStack()
        xt_pool = pa.enter_context(tc.tile_pool(name="xt", bufs=33))
        wq_pool = pa.enter_context(tc.tile_pool(name="wq", bufs=1))

        # DMA descriptor issue, split across engines.  sync: w_qkv + b0
        # nkk0/nkk1; vector: b0 nkk2/nkk3 (before any vector compute).
        wq_sb = wq_pool.tile([128, KC * 3 * HPC * DH], f32r, tag="wq", name="wq")
        nc.sync.dma_start(out=wq_sb, in_=wq_ext.ap())

        xts = {}  # (b, k, nkk) -> [128, 512] tile

        def issue_xt(eng, b, k, nkk):
            t = xt_pool.tile([128, 512], f32r, tag="xt", name=f"xt{b}_{k}_{nkk}")
            eng.dma_start(out=t, in_=xT_ext[b, k][:, nkk * 512:(nkk + 1) * 512])
            xts[(b, k, nkk)] = t

        for nkk in (0, 1):
            for k in range(KC):
                issue_xt(nc.sync, 0, k, nkk)
        for nkk in (2, 3):
            for k in range(KC):
                issue_xt(nc.scalar, 0, k, nkk)

        # ---- constants (after the vector-queue descriptors) ----
        ident_scr = const.tile([128, 128], f32, tag="identS", name="ident_scr")
        make_identity(nc, ident_scr)
        ident = const.tile([128, 128], f32r, tag="ident", name="ident")
        nc.vector.tensor_copy(ident, ident_scr)
        ones2 = const.tile([128, HPC, 1], f32, tag="ones2", name="ones2")
        nc.vector.memset(ones2, 1.0)
        zeros2 = const.tile([128, HPC, 128 - DH - 1], f32, tag="zeros2", name="zeros2")
        nc.vector.memset(zeros2, 0.0)
        # memset on f32r tiles is invalid ISA; zero qT halves via an f32 scratch
        zpad = const.tile([DH, 512], f32, tag="zpad", name="zpad")
        nc.vector.memset(zpad, 0.0)

        def qkv_gen(b):
            """Chain order: k:0 q:0 q:1 v:0+tr k:1 v:1+tr k:2 k:3 q:2 q:3
            v:2+tr v:3+tr.  One yield per matmul / copy group."""
            qT[b] = [
                qk_pool.tile([128, S], f32r, tag="qk", name=f"qT{b}_{h}")
                for h in range(HPC)
            ]
            kT[b] = qk_pool.tile([128, S], f32r, tag="qk", name=f"kT{b}")
            vo[b] = [None] * (S // 128)
            for h in range(HPC):
                r0 = 64 * (1 - h)
                for cq in range(4):
                    nc.vector.tensor_copy(
                        qT[b][h][r0:r0 + 64, cq * 512:(cq + 1) * 512], zpad
                    )
            vT = [None]

            def chain(kind, nkk):
                # kind: 'q' (c0=0), 'k' (c0=1), 'v' (c0=2) column block
                c0 = {"q": 0, "k": 1, "v": 2}[kind] * 128
                ps = ps_a.tile([128, 512], f32, tag="psA", name=f"qkv{b}_{kind}{nkk}")
                for k in range(KC):
                    nc.tensor.matmul(
                        ps,
                        lhsT=wq_sb[:, k * 384 + c0:k * 384 + c0 + 128],
                        rhs=xts[(b, k, nkk)],
                        start=(k == 0),
                        stop=(k == KC - 1),
                    )
                    yield
                cols = slice(nkk * 512, (nkk + 1) * 512)
                if kind == "q":
                    for h in range(HPC):
                        nc.vector.tensor_copy(
                            qT[b][h][64 * h:64 * h + 64, cols], ps[64 * h:64 * h + 64, :]
                        )
                    prog[("q", b)] += 1
                elif kind == "k":
                    nc.vector.tensor_copy(kT[b][:, cols], ps)
                    prog[("k", b)] += 1
                else:
                    if vT[0] is None:
                        vT[0] = vt_pool.tile([128, S], f32r, tag="vt", name=f"vT{b}")
                    nc.vector.tensor_copy(vT[0][:, cols], ps)
                yield

            def vtrans(nkk):
                for sc in range(nkk * 4, nkk * 4 + 4):
                    vps = ps_a.tile([128, 128], f32r, tag="psA", name=f"vps{b}_{sc}")
                    nc.tensor.transpose(vps, vT[0][:, sc * 128:(sc + 1) * 128], ident)
                    vt = vo_pool.tile([128, HPC, 128], f32r, tag="vo", name=f"vo{b}_{sc}")
                    nc.vector.tensor_copy(
                        vt[:, :, 0:DH], vps.rearrange("p (h d) -> p h d", h=HPC)
                    )
                    nc.vector.tensor_copy(vt[:, :, DH:DH + 1], ones2)
                    nc.vector.tensor_copy(vt[:, :, DH + 1:], zeros2)
                    vo[b][sc] = vt
                    prog[("vtr", b)] += 1
                    yield

            yield from chain("k", 0)
            yield from chain("q", 0)
            yield from chain("q", 1)
            yield from chain("v", 0)
            yield from vtrans(0)
            yield from chain("k", 1)
            yield from chain("v", 1)
            yield from vtrans(1)
            yield from chain("k", 2)
            yield from chain("k", 3)
            yield from chain("q", 2)
            yield from chain("q", 3)
            yield from chain("v", 2)
            yield from vtrans(2)
            yield from chain("v", 3)
            yield from vtrans(3)

        state = {"pos": (-1, -1)}  # (global unit index 0..3, qh)

        def pos_gate(u, qh):
            return lambda: state["pos"] >= (u, qh)

        def attention_unit(uidx, b, hh, stage, hooks=None, rate=3):
            h0 = hh * DH
            NK = S // 128
            for qh in range(QH):
                state["pos"] = (uidx, qh)
                q0 = qh * 1024
                weave.pull_until(lambda: prog[("q", b)] >= 2 * (qh + 1))

                def sv(k, st):
                    for half in range(2):
                        nc.tensor.matmul(
                            outT[:, half * 512:(half + 1) * 512],
                            lhsT=vo[b][k][:, hh, :],
                            rhs=st[:, half * 512:(half + 1) * 512],
                            start=(k == 0),
                            stop=(k == NK - 1),
                        )

                outT = ps_ot.tile([128, 1024], f32, tag="psOT", name=f"outT{b}_{hh}_{qh}")
                pending = None
                for k in range(NK):
                    weave.pull_until(lambda: prog[("k", b)] > k // 4)
                    lt = ps_lt.tile([128, 1024], f32, tag="psLT", name=f"lt{b}_{hh}_{qh}_{k}")
                    for half in range(2):
                        nc.tensor.matmul(
                            lt[:, half * 512:(half + 1) * 512],
                            lhsT=kT[b][:, k * 128:(k + 1) * 128],
                            rhs=qT[b][hh][:, q0 + half * 512:q0 + (half + 1) * 512],
                            start=True,
                            stop=True,
                        )
                    st = st_pool.tile([128, 1024], f32r, tag="st", name=f"st{b}_{hh}_{qh}_{k}")
                    nc.scalar.activation(st, lt, EXP)
                    if pending is not None:
                        weave.pull_until(lambda: prog[("vtr", b)] > pending[0])
                        sv(*pending)
                    pending = (k, st)
                    weave.pull(rate)
                weave.pull_until(lambda: prog[("vtr", b)] > pending[0])
                sv(*pending)
                # normalize this q-half straight into the stage tile
                ot_sb = ot_pool.tile([DH, 1024], f32, tag="ot", name=f"ot{b}_{hh}_{qh}")
                nc.vector.tensor_copy(ot_sb, outT[0:DH, :])
                sums = rc_pool.tile([1, 1024], f32, tag="rcp", name=f"sums{b}_{hh}_{qh}")
                nc.vector.tensor_copy(sums, outT[DH:DH + 1, :])
                recip = rc_pool.tile([1, 1024], f32, tag="rcp", name=f"rcp{b}_{hh}_{qh}")
                # approx_fast needs a partition-0-aligned input (an offset-64
                # input returned garbage); ~5x faster than exact reciprocal
                nc.vector.reciprocal_approx_fast(out=recip, in_=sums)
                if dbg and b == 0 and hh == 0 and qh == 0:
                    nc.sync.dma_start(out=dbg["ot"][0:DH], in_=ot_sb)
                    nc.sync.dma_start(out=dbg["ot"][DH:DH + 1], in_=sums)
                bc_sb = bc_pool.tile([DH, 1024], f32, tag="bc", name=f"bc{b}_{hh}_{qh}")
                nc.gpsimd.partition_broadcast(bc_sb, recip)
                if dbg and b == 0 and hh == 0 and qh == 0:
                    nc.sync.dma_start(out=dbg["rc"].ap(), in_=recip)
                    nc.sync.dma_start(out=dbg["bc"].ap(), in_=bc_sb)
                nc.vector.scalar_tensor_tensor(
                    out=stage[h0:h0 + DH, q0:q0 + 1024],
                    in0=ot_sb,
                    scalar=SCALE_INV,
                    in1=bc_sb,
                    op0=mybir.AluOpType.mult,
                    op1=mybir.AluOpType.mult,
                )
                if hooks and qh in hooks:
                    for fn in hooks[qh]:
                        fn()

        def a2a_fire(b, hh, stage):
            h0 = hh * DH
            for qq in range(4):
                nc.gpsimd.dma_start(
                    out=a2a_in[hh, 4 * b + qq],
                    in_=stage[h0:h0 + DH, qq * 512:(qq + 1) * 512],
                )
            nc.gpsimd.collective_compute(
                "AllToAll",
                mybir.AluOpType.bypass,
                replica_groups=[list(range(NCORES))],
                ins=[a2a_in[hh].opt()],
                outs=[a2a_out[b, hh].opt()],
            )

        # ---------- batch 0 ----------
        weave.add(qkv_gen(0))
        weave.add(qkv_gen(1), gate=pos_gate(1, 0))
        # pre-drain: kT[0] nkk0 + qT[0] nkk0/nkk1 (chains 1-3)
        weave.pull_until(lambda: prog[("q", 0)] >= 2)

        stage0 = stage_pool.tile([HPC * DH, S], f32r, tag="stg", name="stg0")

        def xtb1(nkks):
            def fn():
                for nkk in nkks:
                    for k in range(KC):
                        issue_xt(nc.gpsimd, 1, k, nkk)
            return fn

        attention_unit(0, 0, 0, stage0, hooks={0: [xtb1((0,))], 1: [xtb1((1,))]})
        a2a_fire(0, 0, stage0)
        xtb1((2, 3))()
        attention_unit(1, 0, 1, stage0)
        a2a_fire(0, 1, stage0)
        if dbg:
            nc.sync.dma_start(out=dbg["qT"].ap(), in_=qT[0][0][:].bitcast(f32))
            nc.sync.dma_start(out=dbg["kT"].ap(), in_=kT[0][:].bitcast(f32))
            nc.sync.dma_start(out=dbg["stage"].ap(), in_=stage0[:].bitcast(f32))

        # ---------- close phase-A scope, open projection pools ----------
        weave.pull_until(lambda: prog[("vtr", 1)] >= 16)  # finish b1 qkv emission
        pa.close()
        wo_pool = ctx.enter_context(tc.tile_pool(name="wo", bufs=1))
        g_pool = ctx.enter_context(tc.tile_pool(name="g", bufs=10))
        y_pool = ctx.enter_context(tc.tile_pool(name="y", bufs=9))
        bias_pool = ctx.enter_context(tc.tile_pool(name="bias", bufs=1))

        wo_sb = wo_pool.tile([128, KC * D], f32r, tag="wo", name="wo")
        nc.sync.dma_start(out=wo_sb, in_=wo_ext.ap())
        bias_sb = bias_pool.tile([128, D], f32, tag="bias", name="bias_sb")
        bias_ap = bout_ext.ap()
        bias_bcast = bass.AP(
            tensor=bias_ap.tensor,
            offset=bias_ap.offset,
            ap=[[0, 128]] + [list(p) for p in bias_ap.ap],
        )
        nc.sync.dma_start(out=bias_sb, in_=bias_bcast)

        gf_tiles = {}  # (half, k) -> f32r tile

        def g_dma(half, ks):
            for k in ks:
                hh, cc = (0, k) if k < 4 else (1, k - 4)
                t = g_pool.tile([128, 512], f32r, tag="g", name=f"g{half}_{k}")
                nc.gpsimd.dma_start(
                    out=t,
                    in_=a2a_out[half, hh, 2 * cc:2 * cc + 2].rearrange(
                        "s d c -> (s d) c"
                    ),
                )
                gf_tiles[(half, k)] = t

        g_dma(0, range(KC))  # k0-3 dep a2a(0,0) done; k4-7 dep a2a(0,1)

        y_sb = {}  # (half, sc, nk) -> [128, 512] tile

        def proj_phase(half, phase, oext):
            for sc in range(4):
                for nk in range(2):
                    yps = ps_a.tile([128, 512], f32, tag="psA", name=f"yps{half}_{phase}_{sc}_{nk}")
                    for kk in range(4):
                        k = phase * 4 + kk
                        nc.tensor.matmul(
                            yps,
                            lhsT=gf_tiles[(half, k)][:, sc * 128:(sc + 1) * 128],
                            rhs=wo_sb[:, k * D + nk * 512:k * D + (nk + 1) * 512],
                            start=(kk == 0),
                            stop=(kk == 3),
                        )
                        yield
                    if phase == 0:
                        y = y_pool.tile([128, 512], f32, tag="y", name=f"y{half}_{sc}_{nk}")
                        nc.vector.tensor_add(y, yps, bias_sb[:, nk * 512:(nk + 1) * 512])
                        y_sb[(half, sc, nk)] = y
                    else:
                        y = y_sb[(half, sc, nk)]
                        nc.vector.tensor_add(y, y, yps)
                        nc.sync.dma_start(
                            out=oext[sc * 128:(sc + 1) * 128, nk * 512:(nk + 1) * 512],
                            in_=y,
                        )
                    yield

        weave.add(proj_phase(0, 0, outA_ext), gate=pos_gate(2, 1))
        weave.add(proj_phase(0, 1, outA_ext), gate=pos_gate(3, 0))
        weave.add(proj_phase(1, 0, outB_ext), gate=pos_gate(3, 1))

        # ---------- batch 1 ----------
        stage1 = stage_pool.tile([HPC * DH, S], f32r, tag="stg", name="stg1")
        attention_unit(2, 1, 0, stage1)
        if dbg:
            for k in range(KC):
                nc.sync.dma_start(out=dbg["g"][k], in_=gf_tiles[(0, k)][:].bitcast(f32))
        a2a_fire(1, 0, stage1)
        g_dma(1, range(4))
        attention_unit(3, 1, 1, stage1)
        a2a_fire(1, 1, stage1)
        g_dma(1, range(4, 8))

        # ---------- tail: projection B ----------
        state["pos"] = (4, 0)
        while weave.pull(64):
            pass
        # dummy full-array matmuls (no consumer) keep the PE busy through the
        # last AllToAll so the HAM clock stays at 2.4 GHz for phase 2
        for w in range(48):
            wps = ps_lt.tile([128, 512], f32, tag="psLT", name=f"warm{w}")
            nc.tensor.matmul(
                wps,
                lhsT=kT[1][:, 0:128],
                rhs=qT[1][0][:, 0:512],
                start=True,
                stop=True,
            )
        for _ in proj_phase(1, 1, outB_ext):
            pass

    nc.finalize()
    return nc


def _prep_in_maps(x, w_qkv, w_out, b_out):
    x = np.ascontiguousarray(x, dtype=np.float32)
    w_qkv = np.ascontiguousarray(w_qkv, dtype=np.float32)
    w_out = np.ascontiguousarray(w_out, dtype=np.float32)
    b_out = np.ascontiguousarray(b_out, dtype=np.float32)

    xT = np.ascontiguousarray(
        np.stack([x[0].T, x[1].T]).reshape(2, KC, 128, S)
    )
    # arrival order after the per-(batch, head) AllToAll: call h delivers
    # head (2c+h) for c=0..7; stacked [call0 (512 rows), call1 (512 rows)].
    perm = []
    for h in range(HPC):
        for c in range(NCORES):
            base = 128 * c + DH * h
            perm.extend(range(base, base + DH))
    # packed [128, KC, D] so the full w_out is one DMA descriptor
    wo = np.ascontiguousarray(
        w_out[np.array(perm)].reshape(KC, 128, D).transpose(1, 0, 2).reshape(128, KC * D)
    )
    in_maps = []
    for c in range(NCORES):
        c0 = c * HPC * DH
        shard = np.concatenate(
            [
                w_qkv[:, c0:c0 + 128],
                w_qkv[:, D + c0:D + c0 + 128],
                w_qkv[:, 2 * D + c0:2 * D + c0 + 128],
            ],
            axis=1,
        )
        # packed [128, KC, 384] so the per-core w_qkv slice is one descriptor
        wq = np.ascontiguousarray(
            shard.reshape(KC, 128, 384).transpose(1, 0, 2).reshape(128, KC * 384)
        )
        in_maps.append(
            {
                "xT": xT,
                "w_qkv": wq,
                "w_out": wo,
                "b_out": b_out,
            }
        )
    return in_maps


def _run(x, w_qkv, w_out, b_out, trace=False, debug_taps=False):
    _ensure_paths()
    from concourse.bass_utils import run_bass_kernel_spmd

    key = "nc_dbg" if debug_taps else "nc"
    if key not in _CACHE:
        _CACHE[key] = _build_nc(debug_taps=debug_taps)
    nc = _CACHE[key]
    in_maps = _prep_in_maps(x, w_qkv, w_out, b_out)
    res = run_bass_kernel_spmd(nc, in_maps, list(range(NCORES)), trace=trace)
    out = np.empty((2, S, D), dtype=np.float32)
    for c in range(NCORES):
        b, q = c // 4, c % 4
        key = "outA" if b == 0 else "outB"
        out[b, 512 * q:512 * (q + 1), :] = res.results[c][key]
    return out, res


def kernel(x, w_qkv, w_out, b_out):
    out, _ = _run(x, w_qkv, w_out, b_out, trace=False)
    return out
